# revision 9
# baseline (speedup 1.0000x reference)
"""Discounted cumsum (B,H,S,D)=(8,16,4096,128), gamma per head, scan along S.

Strategy: batch-parallel across 8 NeuronCores (1 batch each, all 16 heads).
fp16 I/O with host-side layout permutes so every DMA moves >=512B contiguous
runs (full 360GB/s). Per core the 16 heads split into two engine pipelines:

- SPLIT_M "matmul heads": two-level block scan on the PE. Block length T=127;
  x is host-permuted to [h, i(127), kb(33), d]. An SBUF tile [128, 4224]
  holds 127 x-rows (partitions 0..126) + 1 carry row (partition 127).
  Per head: 8 stage-s matmuls (w^T X -> per-block discounted sums), 1 block
  scan matmul (abt), carry row DMA'd into partition 127, then 9 stage-b
  matmuls A'@X where A' columns 0..126 are the within-block lower-triangular
  decay and column 127 applies the carry (g^(j+1)). Act engine copies
  PSUM->SBUF fp16; results DMA out in the same blocked layout.
- The rest "scan heads": host-transposed to [h, d, s]; one DVE
  tensor_tensor_scan per head (state fp32) does the whole recurrence.

Host un-permutes + upcasts both output groups.
"""
import sys

sys.path.insert(0, "/opt/trn_rl_repo")
import numpy as np

B, H, S, D = 8, 16, 4096, 128
T = 127          # matmul-head block length (127 x rows + 1 carry row)
KB = 33          # blocks per head: 33*127 = 4191 >= 4096
SP = T * KB      # padded sequence length 4191
FM = KB * D      # matmul-head tile free size 4224
SPLIT_M = 10     # heads 0..SPLIT_M-1 on PE, rest on DVE scan

_CACHE = {}


def _build(repeat=1, m=SPLIT_M):
    import contextlib

    import concourse.bacc as bacc
    import concourse.tile as tile
    from concourse import mybir

    f16 = mybir.dt.float16
    f32 = mybir.dt.float32
    ns = H - m
    nsteps = m + 3

    nc = bacc.Bacc("TRN2", target_bir_lowering=False, debug=False)

    xs_in = nc.declare_dram_parameter("xs", [ns, D, S], f16, isOutput=False)
    xm_in = nc.declare_dram_parameter("xm", [m, T, KB, D], f16, isOutput=False)
    g_in = nc.declare_dram_parameter("g", [D, H], f16, isOutput=False)
    at_in = nc.declare_dram_parameter("at", [128, m * T], f16, isOutput=False)
    w_in = nc.declare_dram_parameter("w", [128, m * 65], f16, isOutput=False)
    abt_in = nc.declare_dram_parameter("abt", [32, m * KB], f16, isOutput=False)
    ys_out = nc.declare_dram_parameter("ys", [ns, D, S], f16, isOutput=True)
    ym_out = nc.declare_dram_parameter("ym", [m, T, KB, D], f16, isOutput=True)

    with tile.TileContext(nc) as tc:
        with (
            tc.tile_pool(name="const", bufs=1) as const_pool,
            tc.tile_pool(name="xsp", bufs=3) as xs_pool,
            tc.tile_pool(name="ysp", bufs=3) as ys_pool,
            tc.tile_pool(name="xmp", bufs=4) as xm_pool,
            tc.tile_pool(name="ymp", bufs=2) as ym_pool,
            tc.tile_pool(name="s32", bufs=2) as s32_pool,
            tc.tile_pool(name="c32", bufs=2) as c32_pool,
            tc.tile_pool(name="sps", bufs=2, space="PSUM") as s_psum,
            tc.tile_pool(name="cps", bufs=2, space="PSUM") as c_psum,
            tc.tile_pool(name="yps", bufs=4, space="PSUM") as y_psum,
        ):
            g_sb = const_pool.tile([D, H], f16)
            at_sb = const_pool.tile([128, m * T], f16)
            w_sb = const_pool.tile([128, m * 65], f16)
            abt_sb = const_pool.tile([32, m * KB], f16)
            nc.sync.dma_start(out=g_sb[:], in_=g_in[:])
            nc.sync.dma_start(out=at_sb[:], in_=at_in[:])
            nc.sync.dma_start(out=w_sb[:], in_=w_in[:])
            nc.sync.dma_start(out=abt_sb[:], in_=abt_in[:])

            xm_t = [None] * m

            def m_in(h):
                xm_t[h] = xm_pool.tile([128, FM], f16, name=f"xm{h}", tag="xm")
                dst = xm_t[h][0:T, :].rearrange("p (k d) -> p k d", d=D)
                nc.sync.dma_start(out=dst, in_=xm_in[h])

            def m_s(h):
                # s accumulated directly as [KB, D] in one PSUM bank: matmul k
                # uses a shifted window of wpad whose only nonzero column lands
                # in out row k -> out[k, :] = w^T X_k.
                s_ps = s_psum.tile([KB, D], f32, name="sps", tag="sps")
                for k in range(KB):
                    nc.tensor.matmul(
                        s_ps[:],
                        w_sb[0:T, h * 65 + 32 - k : h * 65 + 65 - k],
                        xm_t[h][0:T, k * D : (k + 1) * D],
                        start=(k == 0),
                        stop=(k == KB - 1),
                    )
                s32 = s32_pool.tile([KB, D], f16, name=f"s32_{h}", tag="s32")
                nc.scalar.copy(out=s32[:], in_=s_ps[:])
                return s32

            s32_t = [None] * m

            def m_c(h):
                c_ps = c_psum.tile([KB, D], f32, name="cps", tag="cps")
                nc.tensor.matmul(
                    c_ps[:],
                    abt_sb[:, h * KB : (h + 1) * KB],
                    s32_t[h][0:32, :],
                    start=True,
                    stop=True,
                )
                c32 = c32_pool.tile([KB, D], f16, name=f"c32_{h}", tag="c32")
                nc.scalar.copy(out=c32[:], in_=c_ps[:])
                nc.gpsimd.dma_start(out=xm_t[h][127:128, :], in_=c32[:])

            def m_b(h):
                ym_t = ym_pool.tile([T, FM], f16, name=f"ym{h}", tag="ym")
                for t in range(9):
                    wdt = 512 if t < 8 else 128
                    sl = slice(t * 512, t * 512 + wdt)
                    y_ps = y_psum.tile([T, wdt], f32, name="yps", tag="yps")
                    nc.tensor.matmul(
                        y_ps[:],
                        at_sb[:, h * T : (h + 1) * T],
                        xm_t[h][:, sl],
                        start=True,
                        stop=True,
                    )
                    nc.scalar.copy(out=ym_t[:, sl], in_=y_ps[:])
                dst = ym_out[h]
                nc.scalar.dma_start(
                    out=dst, in_=ym_t[:].rearrange("p (k d) -> p k d", d=D)
                )

            def s_head(j):
                xt = xs_pool.tile([D, S], f16, name=f"xs{j}", tag="xs")
                nc.sync.dma_start(out=xt[:], in_=xs_in[j])
                yt = ys_pool.tile([D, S], f16, name=f"ys{j}", tag="ys")
                nc.vector.tensor_tensor_scan(
                    out=yt[:],
                    data0=g_sb[:, m + j : m + j + 1].broadcast_to([D, S]),
                    data1=xt[:],
                    initial=0.0,
                    op0=mybir.AluOpType.mult,
                    op1=mybir.AluOpType.add,
                )
                nc.scalar.dma_start(out=ys_out[j], in_=yt[:])

            # scan head j emitted at step scan_step[j]
            scan_step = [int(j * nsteps / max(ns, 1)) for j in range(ns)]

            loop = tc.For_i(0, repeat, 1) if repeat > 1 else contextlib.nullcontext()
            with loop:
                for i in range(nsteps):
                    if i < m:
                        m_in(i)
                    if 0 <= i - 1 < m:
                        s32_t[i - 1] = m_s(i - 1)
                    if 0 <= i - 2 < m:
                        m_c(i - 2)
                    if 0 <= i - 3 < m:
                        m_b(i - 3)
                    for j in range(ns):
                        if scan_step[j] == i:
                            s_head(j)

    nc.compile()
    return nc


def _constants(gamma, m=SPLIT_M):
    g = np.asarray(gamma).astype(np.float64)
    i = np.arange(T)
    diff = i[:, None] - i[None, :]  # [j, p]
    at = np.zeros((128, m * T))
    w = np.zeros((128, m * 65))
    abt = np.zeros((32, m * KB))
    k = np.arange(KB)
    kdiff = k[None, :] - np.arange(32)[:, None] - 1  # [j, k] -> k-1-j
    for h in range(m):
        gh = g[h]
        a_h = np.where(diff >= 0, gh ** np.maximum(diff, 0), 0.0)  # [j, p]
        at[0:T, h * T : (h + 1) * T] = a_h.T
        at[127, h * T : (h + 1) * T] = gh ** (i + 1)
        w[0:T, h * 65 + 32] = gh ** (T - 1 - i)
        G = gh**T
        abt[:, h * KB : (h + 1) * KB] = np.where(
            kdiff >= 0, G ** np.maximum(kdiff, 0), 0.0
        )
    return (
        at.astype(np.float16),
        w.astype(np.float16),
        abt.astype(np.float16),
    )


def _make_in_maps(tensor, gamma, m=SPLIT_M):
    """Full inputs -> per-core input maps (host-side shard + layout)."""
    x16 = np.asarray(tensor, dtype=np.float16)
    g_bc = np.tile(np.asarray(gamma, dtype=np.float16)[None, :], (D, 1))
    at, w, abt = _constants(gamma, m)
    pad = np.zeros((m, SP - S, D), np.float16)
    maps = []
    for c in range(B):
        xm = (
            np.concatenate([x16[c, :m], pad], axis=1)
            .reshape(m, KB, T, D)
            .transpose(0, 2, 1, 3)
        )
        maps.append(
            {
                "xs": np.ascontiguousarray(x16[c, m:].transpose(0, 2, 1)),
                "xm": np.ascontiguousarray(xm),
                "g": g_bc,
                "at": at,
                "w": w,
                "abt": abt,
            }
        )
    return maps


def _postprocess(results, m=SPLIT_M):
    """Per-core {ys, ym} -> full [B, H, S, D] f32."""
    out = np.empty((B, H, S, D), np.float32)
    for c in range(B):
        ym = results[c]["ym"]  # [m, T, KB, D] fp16
        ys = results[c]["ys"]  # [ns, D, S] fp16
        out[c, :m] = (
            ym.transpose(0, 2, 1, 3).reshape(m, SP, D)[:, :S].astype(np.float32)
        )
        out[c, m:] = ys.transpose(0, 2, 1).astype(np.float32)
    return out


def _fast_callable(nc):
    """Cached jitted shard_map callable (avoids per-call retrace)."""
    import jax
    from jax.experimental.shard_map import shard_map
    from jax.sharding import Mesh, NamedSharding, PartitionSpec
    from concourse import bass2jax, mybir

    bass2jax.install_neuronx_cc_hook()
    partition_name = nc.partition_id_tensor.name if nc.partition_id_tensor else None
    in_names, out_names, out_avals, zero_outs = [], [], [], []
    for alloc in nc.m.functions[0].allocations:
        if not isinstance(alloc, mybir.MemoryLocationSet):
            continue
        name = alloc.memorylocations[0].name
        if alloc.kind == "ExternalInput":
            if name != partition_name:
                in_names.append(name)
        elif alloc.kind == "ExternalOutput":
            shape = tuple(alloc.tensor_shape)
            dtype = mybir.dt.np(alloc.dtype)
            out_avals.append(jax.core.ShapedArray(shape, dtype))
            out_names.append(name)
            zero_outs.append(np.zeros(shape, dtype))
    n_params = len(in_names)
    all_in = list(in_names) + list(out_names)
    if partition_name is not None:
        all_in.append(partition_name)

    def _body(*args):
        operands = list(args)
        if partition_name is not None:
            operands.append(bass2jax.partition_id_tensor())
        return tuple(
            bass2jax._bass_exec_p.bind(
                *operands,
                out_avals=tuple(out_avals),
                in_names=tuple(all_in),
                out_names=tuple(out_names),
                lowering_input_output_aliases=(),
                sim_require_finite=True,
                sim_require_nnan=True,
                nc=nc,
            )
        )

    devices = jax.devices()[:B]
    mesh = Mesh(np.asarray(devices), ("core",))
    specs = (PartitionSpec("core"),)
    f = jax.jit(
        shard_map(
            _body,
            mesh=mesh,
            in_specs=specs * (n_params + len(out_names)),
            out_specs=specs * len(out_names),
            check_rep=False,
        ),
        keep_unused=True,
    )
    sharding = NamedSharding(mesh, PartitionSpec("core"))
    dev_zero = [
        jax.device_put(np.zeros((B * z.shape[0], *z.shape[1:]), z.dtype), sharding)
        for z in zero_outs
    ]
    return f, in_names, out_names, out_avals, sharding, dev_zero


def _run_fast(nc, in_maps):
    import jax

    if "fast" not in _CACHE:
        _CACHE["fast"] = _fast_callable(nc)
    f, in_names, out_names, out_avals, sharding, dev_zero = _CACHE["fast"]
    concat_in = [
        jax.device_put(
            np.concatenate([np.asarray(m[nm]) for m in in_maps], axis=0), sharding
        )
        for nm in in_names
    ]
    outs = f(*concat_in, *dev_zero)
    return [
        {
            nm: np.asarray(outs[i]).reshape(B, *out_avals[i].shape)[c]
            for i, nm in enumerate(out_names)
        }
        for c in range(B)
    ]


def _run(tensor, gamma, trace=False, repeat=1):
    from concourse.bass_utils import run_bass_kernel_spmd

    key = f"nc{repeat}"
    if key not in _CACHE:
        _CACHE[key] = _build(repeat)
    nc = _CACHE[key]

    in_maps = _make_in_maps(tensor, gamma)
    if repeat == 1 and not trace:
        try:
            results = _run_fast(nc, in_maps)
            y = _postprocess(results)
            return y, None
        except Exception:
            pass  # fall back to the reference path below
    res = run_bass_kernel_spmd(nc, in_maps, core_ids=list(range(B)), trace=trace)
    y = _postprocess(res.results)
    return y, res


def kernel(tensor, gamma):
    try:
        y, _ = _run(tensor, gamma)
    except Exception:
        # transient device/pool errors: clear cached state and retry once
        _CACHE.clear()
        y, _ = _run(tensor, gamma)
    return y


# revision 13
# speedup vs baseline: 6.6731x; 6.6731x over previous
"""Discounted cumsum (B,H,S,D)=(8,16,4096,128), gamma per head, scan along S.

Strategy: batch-parallel across 8 NeuronCores (1 batch each, all 16 heads).
fp16 I/O with host-side layout permutes so every DMA moves >=512B contiguous
runs (full 360GB/s). Per core the 16 heads split into two engine pipelines:

- SPLIT_M "matmul heads": two-level block scan on the PE. Block length T=127;
  x is host-permuted to [h, i(127), kb(33), d]. An SBUF tile [128, 4224]
  holds 127 x-rows (partitions 0..126) + 1 carry row (partition 127).
  Per head: 8 stage-s matmuls (w^T X -> per-block discounted sums), 1 block
  scan matmul (abt), carry row DMA'd into partition 127, then 9 stage-b
  matmuls A'@X where A' columns 0..126 are the within-block lower-triangular
  decay and column 127 applies the carry (g^(j+1)). Act engine copies
  PSUM->SBUF fp16; results DMA out in the same blocked layout.
- The rest "scan heads": host-transposed to [h, d, s]; one DVE
  tensor_tensor_scan per head (state fp32) does the whole recurrence.

Host un-permutes + upcasts both output groups.
"""
import sys

sys.path.insert(0, "/opt/trn_rl_repo")
import numpy as np

B, H, S, D = 8, 16, 4096, 128
T = 127          # matmul-head block length (127 x rows + 1 carry row)
KB = 33          # blocks per head: 33*127 = 4191 >= 4096
SP = T * KB      # padded sequence length 4191
FM = KB * D      # matmul-head tile free size 4224
SPLIT_M = 10     # heads 0..SPLIT_M-1 on PE, rest on DVE scan

_CACHE = {}


def _build(repeat=1, m=SPLIT_M):
    import contextlib

    import concourse.bacc as bacc
    import concourse.tile as tile
    from concourse import mybir

    f16 = mybir.dt.float16
    f32 = mybir.dt.float32
    ns = H - m
    nsteps = m + 3

    nc = bacc.Bacc("TRN2", target_bir_lowering=False, debug=False)

    xs_in = nc.declare_dram_parameter("xs", [ns, D, S], f16, isOutput=False)
    xm_in = nc.declare_dram_parameter("xm", [m, 128, KB, D], f16, isOutput=False)
    g_in = nc.declare_dram_parameter("g", [D, H], f16, isOutput=False)
    at_in = nc.declare_dram_parameter("at", [128, m * T], f16, isOutput=False)
    w_in = nc.declare_dram_parameter("w", [128, m * 65], f16, isOutput=False)
    abt_in = nc.declare_dram_parameter("abt", [32, m * KB], f16, isOutput=False)
    ys_out = nc.declare_dram_parameter("ys", [ns, D, S], f16, isOutput=True)
    ym_out = nc.declare_dram_parameter("ym", [m, 128, KB, D], f16, isOutput=True)

    with tile.TileContext(nc) as tc:
        with (
            tc.tile_pool(name="const", bufs=1) as const_pool,
            tc.tile_pool(name="xsp", bufs=3) as xs_pool,
            tc.tile_pool(name="ysp", bufs=3) as ys_pool,
            tc.tile_pool(name="xmp", bufs=4) as xm_pool,
            tc.tile_pool(name="ymp", bufs=2) as ym_pool,
            tc.tile_pool(name="s32", bufs=2) as s32_pool,
            tc.tile_pool(name="c32", bufs=2) as c32_pool,
            tc.tile_pool(name="sps", bufs=2, space="PSUM") as s_psum,
            tc.tile_pool(name="cps", bufs=2, space="PSUM") as c_psum,
            tc.tile_pool(name="yps", bufs=4, space="PSUM") as y_psum,
        ):
            g_sb = const_pool.tile([D, H], f16)
            at_sb = const_pool.tile([128, m * T], f16)
            w_sb = const_pool.tile([128, m * 65], f16)
            abt_sb = const_pool.tile([32, m * KB], f16)
            nc.sync.dma_start(out=g_sb[:], in_=g_in[:])
            nc.sync.dma_start(out=at_sb[:], in_=at_in[:])
            nc.sync.dma_start(out=w_sb[:], in_=w_in[:])
            nc.sync.dma_start(out=abt_sb[:], in_=abt_in[:])

            xm_t = [None] * m

            def m_in(h):
                xm_t[h] = xm_pool.tile([128, FM], f16, name=f"xm{h}", tag="xm")
                nc.sync.dma_start(
                    out=xm_t[h][:],
                    in_=xm_in[h].rearrange("p k d -> p (k d)"),
                )

            def m_s(h):
                # s accumulated directly as [KB, D] in one PSUM bank: matmul k
                # uses a shifted window of wpad whose only nonzero column lands
                # in out row k -> out[k, :] = w^T X_k.
                s_ps = s_psum.tile([KB, D], f32, name="sps", tag="sps")
                for k in range(KB):
                    nc.tensor.matmul(
                        s_ps[:],
                        w_sb[0:T, h * 65 + 32 - k : h * 65 + 65 - k],
                        xm_t[h][0:T, k * D : (k + 1) * D],
                        start=(k == 0),
                        stop=(k == KB - 1),
                    )
                s32 = s32_pool.tile([KB, D], f16, name=f"s32_{h}", tag="s32")
                nc.scalar.copy(out=s32[:], in_=s_ps[:])
                return s32

            s32_t = [None] * m

            def m_c(h):
                c_ps = c_psum.tile([KB, D], f32, name="cps", tag="cps")
                nc.tensor.matmul(
                    c_ps[:],
                    abt_sb[:, h * KB : (h + 1) * KB],
                    s32_t[h][0:32, :],
                    start=True,
                    stop=True,
                )
                c32 = c32_pool.tile([KB, D], f16, name=f"c32_{h}", tag="c32")
                nc.scalar.copy(out=c32[:], in_=c_ps[:])
                nc.gpsimd.dma_start(out=xm_t[h][127:128, :], in_=c32[:])

            def m_b(h):
                ym_t = ym_pool.tile([128, FM], f16, name=f"ym{h}", tag="ym")
                for t in range(9):
                    wdt = 512 if t < 8 else 128
                    sl = slice(t * 512, t * 512 + wdt)
                    y_ps = y_psum.tile([T, wdt], f32, name="yps", tag="yps")
                    nc.tensor.matmul(
                        y_ps[:],
                        at_sb[:, h * T : (h + 1) * T],
                        xm_t[h][:, sl],
                        start=True,
                        stop=True,
                    )
                    nc.scalar.copy(out=ym_t[0:T, sl], in_=y_ps[:])
                nc.scalar.dma_start(
                    out=ym_out[h].rearrange("p k d -> p (k d)"), in_=ym_t[:]
                )

            def s_head(j):
                xt = xs_pool.tile([D, S], f16, name=f"xs{j}", tag="xs")
                nc.sync.dma_start(out=xt[:], in_=xs_in[j])
                yt = ys_pool.tile([D, S], f16, name=f"ys{j}", tag="ys")
                nc.vector.tensor_tensor_scan(
                    out=yt[:],
                    data0=g_sb[:, m + j : m + j + 1].broadcast_to([D, S]),
                    data1=xt[:],
                    initial=0.0,
                    op0=mybir.AluOpType.mult,
                    op1=mybir.AluOpType.add,
                )
                nc.scalar.dma_start(out=ys_out[j], in_=yt[:])

            # scan head j emitted at step scan_step[j]
            scan_step = [int(j * nsteps / max(ns, 1)) for j in range(ns)]

            loop = tc.For_i(0, repeat, 1) if repeat > 1 else contextlib.nullcontext()
            with loop:
                for i in range(nsteps):
                    if i < m:
                        m_in(i)
                    if 0 <= i - 1 < m:
                        s32_t[i - 1] = m_s(i - 1)
                    if 0 <= i - 2 < m:
                        m_c(i - 2)
                    if 0 <= i - 3 < m:
                        m_b(i - 3)
                    for j in range(ns):
                        if scan_step[j] == i:
                            s_head(j)

    nc.compile()
    return nc


def _constants(gamma, m=SPLIT_M):
    g = np.asarray(gamma).astype(np.float64)
    i = np.arange(T)
    diff = i[:, None] - i[None, :]  # [j, p]
    at = np.zeros((128, m * T))
    w = np.zeros((128, m * 65))
    abt = np.zeros((32, m * KB))
    k = np.arange(KB)
    kdiff = k[None, :] - np.arange(32)[:, None] - 1  # [j, k] -> k-1-j
    for h in range(m):
        gh = g[h]
        a_h = np.where(diff >= 0, gh ** np.maximum(diff, 0), 0.0)  # [j, p]
        at[0:T, h * T : (h + 1) * T] = a_h.T
        at[127, h * T : (h + 1) * T] = gh ** (i + 1)
        w[0:T, h * 65 + 32] = gh ** (T - 1 - i)
        G = gh**T
        abt[:, h * KB : (h + 1) * KB] = np.where(
            kdiff >= 0, G ** np.maximum(kdiff, 0), 0.0
        )
    return (
        at.astype(np.float16),
        w.astype(np.float16),
        abt.astype(np.float16),
    )


def _make_in_maps(tensor, gamma, m=SPLIT_M):
    """Full inputs -> per-core input maps (host-side shard + layout)."""
    x16 = np.asarray(tensor, dtype=np.float16)
    g_bc = np.tile(np.asarray(gamma, dtype=np.float16)[None, :], (D, 1))
    at, w, abt = _constants(gamma, m)
    pad = np.zeros((m, SP - S, D), np.float16)
    pad_row = np.zeros((m, 1, KB, D), np.float16)
    maps = []
    for c in range(B):
        xm = (
            np.concatenate([x16[c, :m], pad], axis=1)
            .reshape(m, KB, T, D)
            .transpose(0, 2, 1, 3)
        )
        xm = np.concatenate([xm, pad_row], axis=1)  # [m, 128, KB, D]
        maps.append(
            {
                "xs": np.ascontiguousarray(x16[c, m:].transpose(0, 2, 1)),
                "xm": np.ascontiguousarray(xm),
                "g": g_bc,
                "at": at,
                "w": w,
                "abt": abt,
            }
        )
    return maps


def _postprocess(results, m=SPLIT_M):
    """Per-core {ys, ym} -> full [B, H, S, D] f32."""
    out = np.empty((B, H, S, D), np.float32)
    for c in range(B):
        ym = results[c]["ym"][:, :T]  # [m, T, KB, D] fp16 (drop pad row)
        ys = results[c]["ys"]  # [ns, D, S] fp16
        out[c, :m] = (
            ym.transpose(0, 2, 1, 3).reshape(m, SP, D)[:, :S].astype(np.float32)
        )
        out[c, m:] = ys.transpose(0, 2, 1).astype(np.float32)
    return out


def _fast_callable(nc):
    """Cached jitted shard_map callable (avoids per-call retrace)."""
    import jax
    from jax.experimental.shard_map import shard_map
    from jax.sharding import Mesh, NamedSharding, PartitionSpec
    from concourse import bass2jax, mybir

    bass2jax.install_neuronx_cc_hook()
    partition_name = nc.partition_id_tensor.name if nc.partition_id_tensor else None
    in_names, out_names, out_avals, zero_outs = [], [], [], []
    for alloc in nc.m.functions[0].allocations:
        if not isinstance(alloc, mybir.MemoryLocationSet):
            continue
        name = alloc.memorylocations[0].name
        if alloc.kind == "ExternalInput":
            if name != partition_name:
                in_names.append(name)
        elif alloc.kind == "ExternalOutput":
            shape = tuple(alloc.tensor_shape)
            dtype = mybir.dt.np(alloc.dtype)
            out_avals.append(jax.core.ShapedArray(shape, dtype))
            out_names.append(name)
            zero_outs.append(np.zeros(shape, dtype))
    n_params = len(in_names)
    all_in = list(in_names) + list(out_names)
    if partition_name is not None:
        all_in.append(partition_name)

    def _body(*args):
        operands = list(args)
        if partition_name is not None:
            operands.append(bass2jax.partition_id_tensor())
        return tuple(
            bass2jax._bass_exec_p.bind(
                *operands,
                out_avals=tuple(out_avals),
                in_names=tuple(all_in),
                out_names=tuple(out_names),
                lowering_input_output_aliases=(),
                sim_require_finite=True,
                sim_require_nnan=True,
                nc=nc,
            )
        )

    devices = jax.devices()[:B]
    mesh = Mesh(np.asarray(devices), ("core",))
    specs = (PartitionSpec("core"),)
    f = jax.jit(
        shard_map(
            _body,
            mesh=mesh,
            in_specs=specs * (n_params + len(out_names)),
            out_specs=specs * len(out_names),
            check_rep=False,
        ),
        keep_unused=True,
    )
    sharding = NamedSharding(mesh, PartitionSpec("core"))
    dev_zero = [
        jax.device_put(np.zeros((B * z.shape[0], *z.shape[1:]), z.dtype), sharding)
        for z in zero_outs
    ]
    return f, in_names, out_names, out_avals, sharding, dev_zero


def _run_fast(nc, in_maps):
    import jax

    if "fast" not in _CACHE:
        _CACHE["fast"] = _fast_callable(nc)
    f, in_names, out_names, out_avals, sharding, dev_zero = _CACHE["fast"]
    concat_in = [
        jax.device_put(
            np.concatenate([np.asarray(m[nm]) for m in in_maps], axis=0), sharding
        )
        for nm in in_names
    ]
    outs = f(*concat_in, *dev_zero)
    return [
        {
            nm: np.asarray(outs[i]).reshape(B, *out_avals[i].shape)[c]
            for i, nm in enumerate(out_names)
        }
        for c in range(B)
    ]


def _run(tensor, gamma, trace=False, repeat=1):
    from concourse.bass_utils import run_bass_kernel_spmd

    key = f"nc{repeat}"
    if key not in _CACHE:
        _CACHE[key] = _build(repeat)
    nc = _CACHE[key]

    in_maps = _make_in_maps(tensor, gamma)
    if repeat == 1 and not trace:
        try:
            results = _run_fast(nc, in_maps)
            y = _postprocess(results)
            return y, None
        except Exception:
            pass  # fall back to the reference path below
    res = run_bass_kernel_spmd(nc, in_maps, core_ids=list(range(B)), trace=trace)
    y = _postprocess(res.results)
    return y, res


def kernel(tensor, gamma):
    try:
        y, _ = _run(tensor, gamma)
    except Exception:
        # transient device/pool errors: clear cached state and retry once
        _CACHE.clear()
        y, _ = _run(tensor, gamma)
    return y


# revision 15
# speedup vs baseline: 11.0321x; 1.6532x over previous
"""Discounted cumsum (B,H,S,D)=(8,16,4096,128), gamma per head, scan along S.

Strategy: batch-parallel across 8 NeuronCores (1 batch each, all 16 heads).
fp16 I/O with host-side layout permutes so every DMA moves >=512B contiguous
runs (full 360GB/s). Per core the 16 heads split into two engine pipelines:

- SPLIT_M "matmul heads": two-level block scan on the PE. Block length T=127;
  x is host-permuted to [h, i(127), kb(33), d]. An SBUF tile [128, 4224]
  holds 127 x-rows (partitions 0..126) + 1 carry row (partition 127).
  Per head: 8 stage-s matmuls (w^T X -> per-block discounted sums), 1 block
  scan matmul (abt), carry row DMA'd into partition 127, then 9 stage-b
  matmuls A'@X where A' columns 0..126 are the within-block lower-triangular
  decay and column 127 applies the carry (g^(j+1)). Act engine copies
  PSUM->SBUF fp16; results DMA out in the same blocked layout.
- The rest "scan heads": host-transposed to [h, d, s]; one DVE
  tensor_tensor_scan per head (state fp32) does the whole recurrence.

Host un-permutes + upcasts both output groups.
"""
import sys

sys.path.insert(0, "/opt/trn_rl_repo")
import numpy as np

B, H, S, D = 8, 16, 4096, 128
T = 127          # matmul-head block length (127 x rows + 1 carry row)
KB = 33          # blocks per head: 33*127 = 4191 >= 4096
SP = T * KB      # padded sequence length 4191
FM = KB * D      # matmul-head tile free size 4224
SPLIT_M = 10     # heads 0..SPLIT_M-1 on PE, rest on DVE scan

# pool depths (tunable): xm/ym matmul-head tiles, xs/ys scan-head tiles
TUNE = {"xmp": 6, "ymp": 3, "xsp": 4, "ysp": 4}

_CACHE = {}


def _build(repeat=1, m=SPLIT_M):
    import contextlib

    import concourse.bacc as bacc
    import concourse.tile as tile
    from concourse import mybir

    f16 = mybir.dt.float16
    f32 = mybir.dt.float32
    ns = H - m
    nsteps = m + 3

    nc = bacc.Bacc("TRN2", target_bir_lowering=False, debug=False)

    xs_in = nc.declare_dram_parameter("xs", [ns, D, S], f16, isOutput=False)
    xm_in = nc.declare_dram_parameter("xm", [m, 128, KB, D], f16, isOutput=False)
    g_in = nc.declare_dram_parameter("g", [D, H], f16, isOutput=False)
    at_in = nc.declare_dram_parameter("at", [128, m * T], f16, isOutput=False)
    w_in = nc.declare_dram_parameter("w", [128, m * 65], f16, isOutput=False)
    abt_in = nc.declare_dram_parameter("abt", [32, m * KB], f16, isOutput=False)
    ys_out = nc.declare_dram_parameter("ys", [ns, D, S], f16, isOutput=True)
    ym_out = nc.declare_dram_parameter("ym", [m, 128, KB, D], f16, isOutput=True)

    with tile.TileContext(nc) as tc:
        with (
            tc.tile_pool(name="const", bufs=1) as const_pool,
            tc.tile_pool(name="xsp", bufs=TUNE["xsp"]) as xs_pool,
            tc.tile_pool(name="ysp", bufs=TUNE["ysp"]) as ys_pool,
            tc.tile_pool(name="xmp", bufs=TUNE["xmp"]) as xm_pool,
            tc.tile_pool(name="ymp", bufs=TUNE["ymp"]) as ym_pool,
            tc.tile_pool(name="s32", bufs=2) as s32_pool,
            tc.tile_pool(name="c32", bufs=2) as c32_pool,
            tc.tile_pool(name="sps", bufs=2, space="PSUM") as s_psum,
            tc.tile_pool(name="cps", bufs=2, space="PSUM") as c_psum,
            tc.tile_pool(name="yps", bufs=4, space="PSUM") as y_psum,
        ):
            g_sb = const_pool.tile([D, H], f16)
            at_sb = const_pool.tile([128, m * T], f16)
            w_sb = const_pool.tile([128, m * 65], f16)
            abt_sb = const_pool.tile([32, m * KB], f16)
            nc.sync.dma_start(out=g_sb[:], in_=g_in[:])
            nc.sync.dma_start(out=at_sb[:], in_=at_in[:])
            nc.sync.dma_start(out=w_sb[:], in_=w_in[:])
            nc.sync.dma_start(out=abt_sb[:], in_=abt_in[:])

            xm_t = [None] * m

            def m_in(h):
                xm_t[h] = xm_pool.tile([128, FM], f16, name=f"xm{h}", tag="xm")
                nc.sync.dma_start(
                    out=xm_t[h][:],
                    in_=xm_in[h].rearrange("p k d -> p (k d)"),
                )

            def m_s(h):
                # s accumulated directly as [KB, D] in one PSUM bank: matmul k
                # uses a shifted window of wpad whose only nonzero column lands
                # in out row k -> out[k, :] = w^T X_k.
                s_ps = s_psum.tile([KB, D], f32, name="sps", tag="sps")
                for k in range(KB):
                    nc.tensor.matmul(
                        s_ps[:],
                        w_sb[0:T, h * 65 + 32 - k : h * 65 + 65 - k],
                        xm_t[h][0:T, k * D : (k + 1) * D],
                        start=(k == 0),
                        stop=(k == KB - 1),
                    )
                s32 = s32_pool.tile([KB, D], f16, name=f"s32_{h}", tag="s32")
                nc.scalar.copy(out=s32[:], in_=s_ps[:])
                return s32

            s32_t = [None] * m

            def m_c(h):
                c_ps = c_psum.tile([KB, D], f32, name="cps", tag="cps")
                nc.tensor.matmul(
                    c_ps[:],
                    abt_sb[:, h * KB : (h + 1) * KB],
                    s32_t[h][0:32, :],
                    start=True,
                    stop=True,
                )
                c32 = c32_pool.tile([KB, D], f16, name=f"c32_{h}", tag="c32")
                nc.scalar.copy(out=c32[:], in_=c_ps[:])
                nc.gpsimd.dma_start(out=xm_t[h][127:128, :], in_=c32[:])

            def m_b(h):
                ym_t = ym_pool.tile([128, FM], f16, name=f"ym{h}", tag="ym")
                for t in range(9):
                    wdt = 512 if t < 8 else 128
                    sl = slice(t * 512, t * 512 + wdt)
                    y_ps = y_psum.tile([T, wdt], f32, name="yps", tag="yps")
                    nc.tensor.matmul(
                        y_ps[:],
                        at_sb[:, h * T : (h + 1) * T],
                        xm_t[h][:, sl],
                        start=True,
                        stop=True,
                    )
                    nc.scalar.copy(out=ym_t[0:T, sl], in_=y_ps[:])
                nc.scalar.dma_start(
                    out=ym_out[h].rearrange("p k d -> p (k d)"), in_=ym_t[:]
                )

            def s_head(j):
                xt = xs_pool.tile([D, S], f16, name=f"xs{j}", tag="xs")
                nc.sync.dma_start(out=xt[:], in_=xs_in[j])
                yt = ys_pool.tile([D, S], f16, name=f"ys{j}", tag="ys")
                nc.vector.tensor_tensor_scan(
                    out=yt[:],
                    data0=g_sb[:, m + j : m + j + 1].broadcast_to([D, S]),
                    data1=xt[:],
                    initial=0.0,
                    op0=mybir.AluOpType.mult,
                    op1=mybir.AluOpType.add,
                )
                nc.scalar.dma_start(out=ys_out[j], in_=yt[:])

            # scan head j emitted at step scan_step[j]
            scan_step = [int(j * nsteps / max(ns, 1)) for j in range(ns)]

            loop = tc.For_i(0, repeat, 1) if repeat > 1 else contextlib.nullcontext()
            with loop:
                for i in range(nsteps):
                    if i < m:
                        m_in(i)
                    if 0 <= i - 1 < m:
                        s32_t[i - 1] = m_s(i - 1)
                    if 0 <= i - 2 < m:
                        m_c(i - 2)
                    if 0 <= i - 3 < m:
                        m_b(i - 3)
                    for j in range(ns):
                        if scan_step[j] == i:
                            s_head(j)

    nc.compile()
    return nc


def _constants(gamma, m=SPLIT_M):
    g = np.asarray(gamma).astype(np.float64)
    i = np.arange(T)
    diff = i[:, None] - i[None, :]  # [j, p]
    at = np.zeros((128, m * T))
    w = np.zeros((128, m * 65))
    abt = np.zeros((32, m * KB))
    k = np.arange(KB)
    kdiff = k[None, :] - np.arange(32)[:, None] - 1  # [j, k] -> k-1-j
    for h in range(m):
        gh = g[h]
        a_h = np.where(diff >= 0, gh ** np.maximum(diff, 0), 0.0)  # [j, p]
        at[0:T, h * T : (h + 1) * T] = a_h.T
        at[127, h * T : (h + 1) * T] = gh ** (i + 1)
        w[0:T, h * 65 + 32] = gh ** (T - 1 - i)
        G = gh**T
        abt[:, h * KB : (h + 1) * KB] = np.where(
            kdiff >= 0, G ** np.maximum(kdiff, 0), 0.0
        )
    return (
        at.astype(np.float16),
        w.astype(np.float16),
        abt.astype(np.float16),
    )


def _make_in_maps(tensor, gamma, m=SPLIT_M):
    """Full inputs -> per-core input maps (host-side shard + layout)."""
    x16 = np.asarray(tensor, dtype=np.float16)
    g_bc = np.tile(np.asarray(gamma, dtype=np.float16)[None, :], (D, 1))
    at, w, abt = _constants(gamma, m)
    pad = np.zeros((m, SP - S, D), np.float16)
    pad_row = np.zeros((m, 1, KB, D), np.float16)
    maps = []
    for c in range(B):
        xm = (
            np.concatenate([x16[c, :m], pad], axis=1)
            .reshape(m, KB, T, D)
            .transpose(0, 2, 1, 3)
        )
        xm = np.concatenate([xm, pad_row], axis=1)  # [m, 128, KB, D]
        maps.append(
            {
                "xs": np.ascontiguousarray(x16[c, m:].transpose(0, 2, 1)),
                "xm": np.ascontiguousarray(xm),
                "g": g_bc,
                "at": at,
                "w": w,
                "abt": abt,
            }
        )
    return maps


def _postprocess(results, m=SPLIT_M):
    """Per-core {ys, ym} -> full [B, H, S, D] f32."""
    out = np.empty((B, H, S, D), np.float32)
    for c in range(B):
        ym = results[c]["ym"][:, :T]  # [m, T, KB, D] fp16 (drop pad row)
        ys = results[c]["ys"]  # [ns, D, S] fp16
        out[c, :m] = (
            ym.transpose(0, 2, 1, 3).reshape(m, SP, D)[:, :S].astype(np.float32)
        )
        out[c, m:] = ys.transpose(0, 2, 1).astype(np.float32)
    return out


def _fast_callable(nc):
    """Cached jitted shard_map callable (avoids per-call retrace)."""
    import jax
    from jax.experimental.shard_map import shard_map
    from jax.sharding import Mesh, NamedSharding, PartitionSpec
    from concourse import bass2jax, mybir

    bass2jax.install_neuronx_cc_hook()
    partition_name = nc.partition_id_tensor.name if nc.partition_id_tensor else None
    in_names, out_names, out_avals, zero_outs = [], [], [], []
    for alloc in nc.m.functions[0].allocations:
        if not isinstance(alloc, mybir.MemoryLocationSet):
            continue
        name = alloc.memorylocations[0].name
        if alloc.kind == "ExternalInput":
            if name != partition_name:
                in_names.append(name)
        elif alloc.kind == "ExternalOutput":
            shape = tuple(alloc.tensor_shape)
            dtype = mybir.dt.np(alloc.dtype)
            out_avals.append(jax.core.ShapedArray(shape, dtype))
            out_names.append(name)
            zero_outs.append(np.zeros(shape, dtype))
    n_params = len(in_names)
    all_in = list(in_names) + list(out_names)
    if partition_name is not None:
        all_in.append(partition_name)

    def _body(*args):
        operands = list(args)
        if partition_name is not None:
            operands.append(bass2jax.partition_id_tensor())
        return tuple(
            bass2jax._bass_exec_p.bind(
                *operands,
                out_avals=tuple(out_avals),
                in_names=tuple(all_in),
                out_names=tuple(out_names),
                lowering_input_output_aliases=(),
                sim_require_finite=True,
                sim_require_nnan=True,
                nc=nc,
            )
        )

    devices = jax.devices()[:B]
    mesh = Mesh(np.asarray(devices), ("core",))
    specs = (PartitionSpec("core"),)
    f = jax.jit(
        shard_map(
            _body,
            mesh=mesh,
            in_specs=specs * (n_params + len(out_names)),
            out_specs=specs * len(out_names),
            check_rep=False,
        ),
        keep_unused=True,
    )
    sharding = NamedSharding(mesh, PartitionSpec("core"))
    dev_zero = [
        jax.device_put(np.zeros((B * z.shape[0], *z.shape[1:]), z.dtype), sharding)
        for z in zero_outs
    ]
    return f, in_names, out_names, out_avals, sharding, dev_zero


def _run_fast(nc, in_maps):
    import jax

    if "fast" not in _CACHE:
        _CACHE["fast"] = _fast_callable(nc)
    f, in_names, out_names, out_avals, sharding, dev_zero = _CACHE["fast"]
    concat_in = [
        jax.device_put(
            np.concatenate([np.asarray(m[nm]) for m in in_maps], axis=0), sharding
        )
        for nm in in_names
    ]
    outs = f(*concat_in, *dev_zero)
    return [
        {
            nm: np.asarray(outs[i]).reshape(B, *out_avals[i].shape)[c]
            for i, nm in enumerate(out_names)
        }
        for c in range(B)
    ]


def _run(tensor, gamma, trace=False, repeat=1):
    from concourse.bass_utils import run_bass_kernel_spmd

    key = f"nc{repeat}"
    if key not in _CACHE:
        _CACHE[key] = _build(repeat)
    nc = _CACHE[key]

    in_maps = _make_in_maps(tensor, gamma)
    if repeat == 1 and not trace:
        try:
            results = _run_fast(nc, in_maps)
            y = _postprocess(results)
            return y, None
        except Exception:
            pass  # fall back to the reference path below
    res = run_bass_kernel_spmd(nc, in_maps, core_ids=list(range(B)), trace=trace)
    y = _postprocess(res.results)
    return y, res


def kernel(tensor, gamma):
    try:
        y, _ = _run(tensor, gamma)
    except Exception:
        # transient device/pool errors: clear cached state and retry once
        _CACHE.clear()
        y, _ = _run(tensor, gamma)
    return y


# revision 16
# speedup vs baseline: 12.3726x; 1.1215x over previous
"""Discounted cumsum (B,H,S,D)=(8,16,4096,128), gamma per head, scan along S.

Strategy: batch-parallel across 8 NeuronCores (1 batch each, all 16 heads).
fp16 I/O with host-side layout permutes so every DMA moves >=512B contiguous
runs (full 360GB/s). Per core the 16 heads split into two engine pipelines:

- SPLIT_M "matmul heads": two-level block scan on the PE. Block length T=127;
  x is host-permuted to [h, i(127), kb(33), d]. An SBUF tile [128, 4224]
  holds 127 x-rows (partitions 0..126) + 1 carry row (partition 127).
  Per head: 8 stage-s matmuls (w^T X -> per-block discounted sums), 1 block
  scan matmul (abt), carry row DMA'd into partition 127, then 9 stage-b
  matmuls A'@X where A' columns 0..126 are the within-block lower-triangular
  decay and column 127 applies the carry (g^(j+1)). Act engine copies
  PSUM->SBUF fp16; results DMA out in the same blocked layout.
- The rest "scan heads": host-transposed to [h, d, s]; one DVE
  tensor_tensor_scan per head (state fp32) does the whole recurrence.

Host un-permutes + upcasts both output groups.
"""
import sys

sys.path.insert(0, "/opt/trn_rl_repo")
import numpy as np

B, H, S, D = 8, 16, 4096, 128
T = 127          # matmul-head block length (127 x rows + 1 carry row)
KB = 33          # blocks per head: 33*127 = 4191 >= 4096
SP = T * KB      # padded sequence length 4191
FM = KB * D      # matmul-head tile free size 4224
SPLIT_M = 10     # heads 0..SPLIT_M-1 on PE, rest on DVE scan

# pool depths (tunable): xm/ym matmul-head tiles, xs/ys scan-head tiles
TUNE = {"xmp": 6, "ymp": 3, "xsp": 4, "ysp": 4, "dve_copy_heads": ()}

_CACHE = {}


def _build(repeat=1, m=SPLIT_M):
    import contextlib

    import concourse.bacc as bacc
    import concourse.tile as tile
    from concourse import mybir

    f16 = mybir.dt.float16
    f32 = mybir.dt.float32
    ns = H - m
    nsteps = m + 3

    nc = bacc.Bacc("TRN2", target_bir_lowering=False, debug=False)

    xs_in = nc.declare_dram_parameter("xs", [ns, D, S], f16, isOutput=False)
    xm_in = nc.declare_dram_parameter("xm", [m, 128, KB, D], f16, isOutput=False)
    g_in = nc.declare_dram_parameter("g", [D, H], f16, isOutput=False)
    at_in = nc.declare_dram_parameter("at", [128, m * T], f16, isOutput=False)
    w_in = nc.declare_dram_parameter("w", [128, m * 65], f16, isOutput=False)
    abt_in = nc.declare_dram_parameter("abt", [32, m * KB], f16, isOutput=False)
    ys_out = nc.declare_dram_parameter("ys", [ns, D, S], f16, isOutput=True)
    ym_out = nc.declare_dram_parameter("ym", [m, 128, KB, D], f16, isOutput=True)

    with tile.TileContext(nc) as tc:
        with (
            tc.tile_pool(name="const", bufs=1) as const_pool,
            tc.tile_pool(name="xsp", bufs=TUNE["xsp"]) as xs_pool,
            tc.tile_pool(name="ysp", bufs=TUNE["ysp"]) as ys_pool,
            tc.tile_pool(name="xmp", bufs=TUNE["xmp"]) as xm_pool,
            tc.tile_pool(name="ymp", bufs=TUNE["ymp"]) as ym_pool,
            tc.tile_pool(name="s32", bufs=2) as s32_pool,
            tc.tile_pool(name="c32", bufs=2) as c32_pool,
            tc.tile_pool(name="sps", bufs=2, space="PSUM") as s_psum,
            tc.tile_pool(name="cps", bufs=2, space="PSUM") as c_psum,
            tc.tile_pool(name="yps", bufs=4, space="PSUM") as y_psum,
        ):
            g_sb = const_pool.tile([D, H], f16)
            at_sb = const_pool.tile([128, m * T], f16)
            w_sb = const_pool.tile([128, m * 65], f16)
            abt_sb = const_pool.tile([32, m * KB], f16)
            nc.sync.dma_start(out=g_sb[:], in_=g_in[:])
            nc.sync.dma_start(out=at_sb[:], in_=at_in[:])
            nc.sync.dma_start(out=w_sb[:], in_=w_in[:])
            nc.sync.dma_start(out=abt_sb[:], in_=abt_in[:])

            xm_t = [None] * m

            def m_in(h):
                xm_t[h] = xm_pool.tile([128, FM], f16, name=f"xm{h}", tag="xm")
                nc.sync.dma_start(
                    out=xm_t[h][:],
                    in_=xm_in[h].rearrange("p k d -> p (k d)"),
                )

            def m_s(h):
                # s accumulated directly as [KB, D] in one PSUM bank: matmul k
                # uses a shifted window of wpad whose only nonzero column lands
                # in out row k -> out[k, :] = w^T X_k.
                s_ps = s_psum.tile([KB, D], f32, name="sps", tag="sps")
                for k in range(KB):
                    nc.tensor.matmul(
                        s_ps[:],
                        w_sb[0:T, h * 65 + 32 - k : h * 65 + 65 - k],
                        xm_t[h][0:T, k * D : (k + 1) * D],
                        start=(k == 0),
                        stop=(k == KB - 1),
                    )
                s32 = s32_pool.tile([KB, D], f16, name=f"s32_{h}", tag="s32")
                nc.scalar.copy(out=s32[:], in_=s_ps[:])
                return s32

            s32_t = [None] * m

            def m_c(h):
                c_ps = c_psum.tile([KB, D], f32, name="cps", tag="cps")
                nc.tensor.matmul(
                    c_ps[:],
                    abt_sb[:, h * KB : (h + 1) * KB],
                    s32_t[h][0:32, :],
                    start=True,
                    stop=True,
                )
                c32 = c32_pool.tile([KB, D], f16, name=f"c32_{h}", tag="c32")
                nc.scalar.copy(out=c32[:], in_=c_ps[:])
                nc.gpsimd.dma_start(out=xm_t[h][127:128, :], in_=c32[:])

            def m_b(h):
                ym_t = ym_pool.tile([128, FM], f16, name=f"ym{h}", tag="ym")
                on_dve = h in TUNE["dve_copy_heads"]
                for t in range(9):
                    wdt = 512 if t < 8 else 128
                    sl = slice(t * 512, t * 512 + wdt)
                    y_ps = y_psum.tile([T, wdt], f32, name="yps", tag="yps")
                    nc.tensor.matmul(
                        y_ps[:],
                        at_sb[:, h * T : (h + 1) * T],
                        xm_t[h][:, sl],
                        start=True,
                        stop=True,
                    )
                    if on_dve:
                        nc.vector.tensor_copy(out=ym_t[0:T, sl], in_=y_ps[:])
                    else:
                        nc.scalar.copy(out=ym_t[0:T, sl], in_=y_ps[:])
                nc.scalar.dma_start(
                    out=ym_out[h].rearrange("p k d -> p (k d)"), in_=ym_t[:]
                )

            def s_head(j):
                xt = xs_pool.tile([D, S], f16, name=f"xs{j}", tag="xs")
                nc.sync.dma_start(out=xt[:], in_=xs_in[j])
                yt = ys_pool.tile([D, S], f16, name=f"ys{j}", tag="ys")
                nc.vector.tensor_tensor_scan(
                    out=yt[:],
                    data0=g_sb[:, m + j : m + j + 1].broadcast_to([D, S]),
                    data1=xt[:],
                    initial=0.0,
                    op0=mybir.AluOpType.mult,
                    op1=mybir.AluOpType.add,
                )
                nc.scalar.dma_start(out=ys_out[j], in_=yt[:])

            # scan head j emitted at step scan_step[j]
            scan_step = [int(j * nsteps / max(ns, 1)) for j in range(ns)]

            loop = tc.For_i(0, repeat, 1) if repeat > 1 else contextlib.nullcontext()
            with loop:
                for i in range(nsteps):
                    if i < m:
                        m_in(i)
                    if 0 <= i - 1 < m:
                        s32_t[i - 1] = m_s(i - 1)
                    if 0 <= i - 2 < m:
                        m_c(i - 2)
                    if 0 <= i - 3 < m:
                        m_b(i - 3)
                    for j in range(ns):
                        if scan_step[j] == i:
                            s_head(j)

    nc.compile()
    return nc


def _constants(gamma, m=SPLIT_M):
    g = np.asarray(gamma).astype(np.float64)
    i = np.arange(T)
    diff = i[:, None] - i[None, :]  # [j, p]
    at = np.zeros((128, m * T))
    w = np.zeros((128, m * 65))
    abt = np.zeros((32, m * KB))
    k = np.arange(KB)
    kdiff = k[None, :] - np.arange(32)[:, None] - 1  # [j, k] -> k-1-j
    for h in range(m):
        gh = g[h]
        a_h = np.where(diff >= 0, gh ** np.maximum(diff, 0), 0.0)  # [j, p]
        at[0:T, h * T : (h + 1) * T] = a_h.T
        at[127, h * T : (h + 1) * T] = gh ** (i + 1)
        w[0:T, h * 65 + 32] = gh ** (T - 1 - i)
        G = gh**T
        abt[:, h * KB : (h + 1) * KB] = np.where(
            kdiff >= 0, G ** np.maximum(kdiff, 0), 0.0
        )
    return (
        at.astype(np.float16),
        w.astype(np.float16),
        abt.astype(np.float16),
    )


def _make_in_maps(tensor, gamma, m=SPLIT_M):
    """Full inputs -> per-core input maps (host-side shard + layout)."""
    x16 = np.asarray(tensor, dtype=np.float16)
    g_bc = np.tile(np.asarray(gamma, dtype=np.float16)[None, :], (D, 1))
    at, w, abt = _constants(gamma, m)
    pad = np.zeros((m, SP - S, D), np.float16)
    pad_row = np.zeros((m, 1, KB, D), np.float16)
    maps = []
    for c in range(B):
        xm = (
            np.concatenate([x16[c, :m], pad], axis=1)
            .reshape(m, KB, T, D)
            .transpose(0, 2, 1, 3)
        )
        xm = np.concatenate([xm, pad_row], axis=1)  # [m, 128, KB, D]
        maps.append(
            {
                "xs": np.ascontiguousarray(x16[c, m:].transpose(0, 2, 1)),
                "xm": np.ascontiguousarray(xm),
                "g": g_bc,
                "at": at,
                "w": w,
                "abt": abt,
            }
        )
    return maps


def _postprocess(results, m=SPLIT_M):
    """Per-core {ys, ym} -> full [B, H, S, D] f32."""
    out = np.empty((B, H, S, D), np.float32)
    for c in range(B):
        ym = results[c]["ym"][:, :T]  # [m, T, KB, D] fp16 (drop pad row)
        ys = results[c]["ys"]  # [ns, D, S] fp16
        out[c, :m] = (
            ym.transpose(0, 2, 1, 3).reshape(m, SP, D)[:, :S].astype(np.float32)
        )
        out[c, m:] = ys.transpose(0, 2, 1).astype(np.float32)
    return out


def _fast_callable(nc):
    """Cached jitted shard_map callable (avoids per-call retrace)."""
    import jax
    from jax.experimental.shard_map import shard_map
    from jax.sharding import Mesh, NamedSharding, PartitionSpec
    from concourse import bass2jax, mybir

    bass2jax.install_neuronx_cc_hook()
    partition_name = nc.partition_id_tensor.name if nc.partition_id_tensor else None
    in_names, out_names, out_avals, zero_outs = [], [], [], []
    for alloc in nc.m.functions[0].allocations:
        if not isinstance(alloc, mybir.MemoryLocationSet):
            continue
        name = alloc.memorylocations[0].name
        if alloc.kind == "ExternalInput":
            if name != partition_name:
                in_names.append(name)
        elif alloc.kind == "ExternalOutput":
            shape = tuple(alloc.tensor_shape)
            dtype = mybir.dt.np(alloc.dtype)
            out_avals.append(jax.core.ShapedArray(shape, dtype))
            out_names.append(name)
            zero_outs.append(np.zeros(shape, dtype))
    n_params = len(in_names)
    all_in = list(in_names) + list(out_names)
    if partition_name is not None:
        all_in.append(partition_name)

    def _body(*args):
        operands = list(args)
        if partition_name is not None:
            operands.append(bass2jax.partition_id_tensor())
        return tuple(
            bass2jax._bass_exec_p.bind(
                *operands,
                out_avals=tuple(out_avals),
                in_names=tuple(all_in),
                out_names=tuple(out_names),
                lowering_input_output_aliases=(),
                sim_require_finite=True,
                sim_require_nnan=True,
                nc=nc,
            )
        )

    devices = jax.devices()[:B]
    mesh = Mesh(np.asarray(devices), ("core",))
    specs = (PartitionSpec("core"),)
    f = jax.jit(
        shard_map(
            _body,
            mesh=mesh,
            in_specs=specs * (n_params + len(out_names)),
            out_specs=specs * len(out_names),
            check_rep=False,
        ),
        keep_unused=True,
    )
    sharding = NamedSharding(mesh, PartitionSpec("core"))
    dev_zero = [
        jax.device_put(np.zeros((B * z.shape[0], *z.shape[1:]), z.dtype), sharding)
        for z in zero_outs
    ]
    return f, in_names, out_names, out_avals, sharding, dev_zero


def _run_fast(nc, in_maps):
    import jax

    if "fast" not in _CACHE:
        _CACHE["fast"] = _fast_callable(nc)
    f, in_names, out_names, out_avals, sharding, dev_zero = _CACHE["fast"]
    concat_in = [
        jax.device_put(
            np.concatenate([np.asarray(m[nm]) for m in in_maps], axis=0), sharding
        )
        for nm in in_names
    ]
    outs = f(*concat_in, *dev_zero)
    return [
        {
            nm: np.asarray(outs[i]).reshape(B, *out_avals[i].shape)[c]
            for i, nm in enumerate(out_names)
        }
        for c in range(B)
    ]


def _run(tensor, gamma, trace=False, repeat=1):
    from concourse.bass_utils import run_bass_kernel_spmd

    key = f"nc{repeat}"
    if key not in _CACHE:
        _CACHE[key] = _build(repeat)
    nc = _CACHE[key]

    in_maps = _make_in_maps(tensor, gamma)
    if repeat == 1 and not trace:
        try:
            results = _run_fast(nc, in_maps)
            y = _postprocess(results)
            return y, None
        except Exception:
            pass  # fall back to the reference path below
    res = run_bass_kernel_spmd(nc, in_maps, core_ids=list(range(B)), trace=trace)
    y = _postprocess(res.results)
    return y, res


def kernel(tensor, gamma):
    try:
        y, _ = _run(tensor, gamma)
    except Exception:
        # transient device/pool errors: clear cached state and retry once
        _CACHE.clear()
        y, _ = _run(tensor, gamma)
    return y
